# revision 2
# baseline (speedup 1.0000x reference)
"""Trainium2 Bass kernel for gnn_message_passing (nn_Conv_82506321756833).

Computes, for N=50000 nodes / E=800000 edges / H=128:
    xp   = gelu(x @ W1 + b1)
    aggr = segment_sum(xp[src] * bases, dst)
    x    = x_feat + aggr
    y    = gelu(bn1(x @ W2 + b2)); y = gelu(bn2(y @ W3 + b3))
    out  = x + y

Sharding: nodes are partitioned contiguously across 8 cores (graph parallel);
each core owns its node shard and all edges whose dst lands in the shard.
Every core redundantly computes the full xp table (cheap GEMM) so the edge
gather is purely local.  The scatter-sum is done on-chip with one-hot
matmuls: edges are bucketed by 128-node destination windows (host-side sort),
and for each 128-edge tile a one-hot matrix S[e, slot] selects the
destination slot; PE computes msg.T @ S accumulating into PSUM (feature-major
output feeds the FFN directly, with BN folded into W2/W3 + bias vectors).

The edge gather uses InstDMAGatherAnt (int16 indices, signed): each window's
edges are split into "lo" (src < 32768) and "hi" tile groups; the hi group
gathers from an offset view of the xp table.
"""

import numpy as np

import concourse.bass as bass
import concourse.bacc as bacc
import concourse.tile as tile
from concourse import mybir
from concourse.bass_utils import run_bass_kernel_spmd

H = 128
WIN = 128
SPLIT = 32768  # int16 index limit for dma_gather
BN_EPS = 1e-5
F16 = mybir.dt.float16
F32 = mybir.dt.float32
I16 = mybir.dt.int16
GELU = mybir.ActivationFunctionType.Gelu


def _ceil_to(x, m):
    return (x + m - 1) // m * m


def _wrap16(idx, nw, L):
    """[nw, L] int16 index lists -> [nw*128, L//16] wrapped+replicated."""
    m = idx.reshape(nw, L // 16, 16).transpose(0, 2, 1)  # [nw, 16, L/16]
    m = np.tile(m, (1, 8, 1))                            # [nw, 128, L/16]
    return np.ascontiguousarray(m.reshape(nw * 128, L // 16))


def prep_inputs(x_feat, bases, src, dst, W1, b1, W2, b2, W3, b3,
                g1, be1, m1, v1, g2, be2, m2, v2, ncores=8):
    """Host-side sharding: bucket edges by (dst window, src-range), sort,
    pad each group to a fixed tile count, build per-core input maps."""
    N = x_feat.shape[0]
    assert N % ncores == 0
    NSH = N // ncores
    NW = (NSH + WIN - 1) // WIN
    NPAD = NW * WIN
    NA = _ceil_to(N, 128)

    x_feat = np.asarray(x_feat, np.float32)
    bases = np.asarray(bases, np.float32)
    src = np.asarray(src, np.int64)
    dst = np.asarray(dst, np.int64)

    xT = np.zeros((H, NA), np.float16)
    xT[:, :N] = x_feat.T

    core_of = dst // NSH
    percore = []
    TLO = THI = 0
    for k in range(ncores):
        sel = np.nonzero(core_of == k)[0]
        ld = dst[sel] - k * NSH
        w = ld // WIN
        j = ld % WIN
        s = src[sel]
        hi = (s >= SPLIT).astype(np.int64)
        key2 = w * 2 + hi
        order = np.lexsort((s, key2))
        w, j, s, hi, key2, sel = (w[order], j[order], s[order], hi[order],
                                  key2[order], sel[order])
        cnt2 = np.bincount(key2, minlength=NW * 2)
        nlo = cnt2[0::2]
        nhi = cnt2[1::2]
        TLO = max(TLO, int(np.max((nlo + 127) // 128)))
        THI = max(THI, int(np.max((nhi + 127) // 128)))
        starts2 = np.zeros(NW * 2, np.int64)
        np.cumsum(cnt2[:-1], out=starts2[1:])
        rank = np.arange(len(w)) - starts2[key2]
        percore.append((w, j, s, hi, rank, sel))
    TLO = max(TLO, 1)
    T = TLO + THI

    w1h = np.ascontiguousarray(np.asarray(W1, np.float32).astype(np.float16))
    a1 = (np.asarray(g1, np.float32) /
          np.sqrt(np.asarray(v1, np.float32) + BN_EPS))
    a2 = (np.asarray(g2, np.float32) /
          np.sqrt(np.asarray(v2, np.float32) + BN_EPS))
    w2f = np.ascontiguousarray((np.asarray(W2, np.float32) * a1[None, :])
                               .astype(np.float16))
    w3f = np.ascontiguousarray((np.asarray(W3, np.float32) * a2[None, :])
                               .astype(np.float16))
    c2 = ((np.asarray(b2, np.float32) - np.asarray(m1, np.float32)) * a1
          + np.asarray(be1, np.float32)).astype(np.float32).reshape(H, 1)
    c3 = ((np.asarray(b3, np.float32) - np.asarray(m2, np.float32)) * a2
          + np.asarray(be2, np.float32)).astype(np.float32).reshape(H, 1)
    have_b1 = bool(np.any(np.asarray(b1)))
    b1h = np.asarray(b1, np.float32).astype(np.float16).reshape(1, H)

    in_maps = []
    for k in range(ncores):
        w, j, s, hi, rank, sel = percore[k]
        # position of each edge inside its window's [T*128] slot grid:
        # lo edges occupy tiles [0, TLO), hi edges tiles [TLO, T).
        pos = np.where(hi == 1, TLO * 128 + rank, rank)
        # dma_gather writes index i -> (partition i%128, block i//128)
        perm = (w * 128 + pos % 128) * T + pos // 128
        bas_all = np.zeros((NW * 128 * T, H), np.float16)
        bas_all[perm] = bases[sel].astype(np.float16)
        s_all = np.zeros((NW * 128 * T, H), np.float16)
        s_all[perm, j] = 1.0

        ilo = np.zeros((NW, TLO * 128), np.int16)
        lo_m = hi == 0
        ilo[w[lo_m], rank[lo_m]] = s[lo_m].astype(np.int16)
        maps = dict(
            xT=xT,
            basd=bas_all.reshape(NW * 128, T * H),
            sd=s_all.reshape(NW * 128, T * H),
            ilod=_wrap16(ilo, NW, TLO * 128),
            w1=w1h, w2=w2f, w3=w3f, c2=c2, c3=c3,
        )
        if THI:
            ihi = np.zeros((NW, THI * 128), np.int16)
            hi_m = hi == 1
            ihi[w[hi_m], rank[hi_m]] = (s[hi_m] - SPLIT).astype(np.int16)
            maps["ihid"] = _wrap16(ihi, NW, THI * 128)
        xfm = np.zeros((H, NPAD), np.float32)
        xfm[:, :NSH] = x_feat[k * NSH:(k + 1) * NSH].T
        maps["xfm"] = xfm
        if have_b1:
            maps["b1"] = b1h
        in_maps.append(maps)
    meta = dict(N=N, NSH=NSH, NW=NW, NPAD=NPAD, NA=NA,
                TLO=TLO, THI=THI, T=T, have_b1=have_b1)
    return in_maps, meta


def build_program(meta, ncores=8, act=GELU):
    NA, NW, NPAD = meta["NA"], meta["NW"], meta["NPAD"]
    TLO, THI, T = meta["TLO"], meta["THI"], meta["T"]
    have_b1 = meta["have_b1"]
    EPW = T * H

    nc = bacc.Bacc("TRN2", target_bir_lowering=False, debug=False,
                   num_devices=ncores)
    xT = nc.dram_tensor("xT", [H, NA], F16, kind="ExternalInput").ap()
    xfm = nc.dram_tensor("xfm", [H, NPAD], F32, kind="ExternalInput").ap()
    basd = nc.dram_tensor("basd", [NW * 128, EPW], F16,
                          kind="ExternalInput").ap()
    sd = nc.dram_tensor("sd", [NW * 128, EPW], F16,
                        kind="ExternalInput").ap()
    ilod = nc.dram_tensor("ilod", [NW * 128, TLO * 8], I16,
                          kind="ExternalInput").ap()
    ihid = (nc.dram_tensor("ihid", [NW * 128, THI * 8], I16,
                           kind="ExternalInput").ap() if THI else None)
    w1 = nc.dram_tensor("w1", [H, H], F16, kind="ExternalInput").ap()
    w2 = nc.dram_tensor("w2", [H, H], F16, kind="ExternalInput").ap()
    w3 = nc.dram_tensor("w3", [H, H], F16, kind="ExternalInput").ap()
    c2 = nc.dram_tensor("c2", [H, 1], F32, kind="ExternalInput").ap()
    c3 = nc.dram_tensor("c3", [H, 1], F32, kind="ExternalInput").ap()
    b1 = (nc.dram_tensor("b1", [1, H], F16, kind="ExternalInput").ap()
          if have_b1 else None)
    outd = nc.dram_tensor("out", [H, NPAD], F32, kind="ExternalOutput").ap()
    xp_h = nc.dram_tensor("xp", [NA, H], F16)
    xp_full = xp_h.ap()

    with tile.TileContext(nc) as tc:
        with (
            tc.tile_pool(name="const", bufs=1) as cpool,
            tc.tile_pool(name="xa", bufs=3) as xa,
            tc.tile_pool(name="xo", bufs=3) as xo,
            tc.tile_pool(name="pa", bufs=2, space="PSUM") as pa,
            tc.tile_pool(name="bas", bufs=3) as basp,
            tc.tile_pool(name="gat", bufs=3) as gatp,
            tc.tile_pool(name="st", bufs=3) as stp,
            tc.tile_pool(name="small", bufs=3) as smallp,
            tc.tile_pool(name="ffn", bufs=2) as ffnp,
            tc.tile_pool(name="pag", bufs=2, space="PSUM") as pag,
            tc.tile_pool(name="pffn", bufs=2, space="PSUM") as pffn,
        ):
            # constants
            w1t = cpool.tile([H, H], F16, tag="w1")
            nc.sync.dma_start(w1t[:], w1[:])
            w2t = cpool.tile([H, H], F16, tag="w2")
            nc.sync.dma_start(w2t[:], w2[:])
            w3t = cpool.tile([H, H], F16, tag="w3")
            nc.sync.dma_start(w3t[:], w3[:])
            c2t = cpool.tile([H, 1], F32, tag="c2")
            nc.sync.dma_start(c2t[:], c2[:])
            c3t = cpool.tile([H, 1], F32, tag="c3")
            nc.sync.dma_start(c3t[:], c3[:])
            if have_b1:
                b1t = cpool.tile([1, H], F16, tag="b1")
                nc.sync.dma_start(b1t[:], b1[:])
                onest = cpool.tile([1, H], F16, tag="ones")
                nc.gpsimd.memset(onest[:], 1.0)

            # ---- Phase A: xp = gelu(x @ W1 [+ b1]), node-major fp16 ----
            CH = 512
            for c0 in range(0, NA, CH):
                cols = min(CH, NA - c0)
                nb = cols // 128
                xt_t = xa.tile([H, CH], F16, tag="xa")
                nc.sync.dma_start(xt_t[:, :cols], xT[:, c0:c0 + cols])
                ps = pa.tile([128, CH], F32, tag="pa")
                for b in range(nb):
                    nc.tensor.matmul(
                        ps[:, b * 128:(b + 1) * 128],
                        xt_t[:, b * 128:(b + 1) * 128],
                        w1t[:],
                        start=True, stop=not have_b1)
                    if have_b1:
                        nc.tensor.matmul(
                            ps[:, b * 128:(b + 1) * 128],
                            onest[:1, :], b1t[:1, :],
                            start=False, stop=True)
                xo_t = xo.tile([128, CH], F16, tag="xo")
                nc.scalar.activation(xo_t[:, :cols], ps[:, :cols], act)
                dst_ap = bass.AP(xp_h, c0 * H,
                                 [[H, 128], [128 * H, nb], [1, H]])
                nc.sync.dma_start(dst_ap, xo_t[:, :cols])

            # ---- Phase B: gather, multiply, one-hot scatter, FFN ----
            for w in range(NW):
                r0 = w * 128
                bas_t = basp.tile([128, EPW], F16, tag="bas")
                nc.scalar.dma_start(bas_t[:], basd[r0:r0 + 128, :])
                il_t = smallp.tile([128, TLO * 8], I16, tag="il")
                nc.scalar.dma_start(il_t[:], ilod[r0:r0 + 128, :])
                if THI:
                    ih_t = smallp.tile([128, THI * 8], I16, tag="ih")
                    nc.scalar.dma_start(ih_t[:], ihid[r0:r0 + 128, :])
                s_t = stp.tile([128, EPW], F16, tag="s")
                nc.sync.dma_start(s_t[:], sd[r0:r0 + 128, :])

                g_t = gatp.tile([128, EPW], F16, tag="gat")
                g3 = g_t[:].rearrange("p (t f) -> p t f", f=H)
                nc.gpsimd.dma_gather(g3[:, 0:TLO, :], xp_full, il_t[:],
                                     TLO * 128, TLO * 128, H,
                                     single_packet=False)
                if THI:
                    nc.gpsimd.dma_gather(g3[:, TLO:T, :],
                                         xp_full[SPLIT:NA, :], ih_t[:],
                                         THI * 128, THI * 128, H,
                                         single_packet=False)
                msg_t = gatp.tile([128, EPW], F16, tag="msg")
                nc.vector.tensor_mul(msg_t[:], g_t[:], bas_t[:])

                ps_ag = pag.tile([128, 128], F32, tag="pag")
                for t in range(T):
                    nc.tensor.matmul(
                        ps_ag[:],
                        msg_t[:, t * 128:(t + 1) * 128],
                        s_t[:, t * 128:(t + 1) * 128],
                        start=(t == 0), stop=(t == T - 1))

                xf_t = smallp.tile([128, 128], F32, tag="xf")
                nc.scalar.dma_start(xf_t[:], xfm[:, r0:r0 + 128])
                x32_t = ffnp.tile([128, 128], F32, tag="x32")
                nc.vector.tensor_add(x32_t[:], ps_ag[:], xf_t[:])
                x16_t = ffnp.tile([128, 128], F16, tag="x16")
                nc.vector.tensor_copy(x16_t[:], x32_t[:])

                ps2 = pffn.tile([128, 128], F32, tag="pffn")
                nc.tensor.matmul(ps2[:], w2t[:], x16_t[:],
                                 start=True, stop=True)
                y1_t = ffnp.tile([128, 128], F16, tag="y1")
                nc.scalar.activation(y1_t[:], ps2[:], act, bias=c2t[:, 0:1])
                ps3 = pffn.tile([128, 128], F32, tag="pffn")
                nc.tensor.matmul(ps3[:], w3t[:], y1_t[:],
                                 start=True, stop=True)
                y2_t = ffnp.tile([128, 128], F32, tag="y2")
                nc.scalar.activation(y2_t[:], ps3[:], act, bias=c3t[:, 0:1])
                o_t = ffnp.tile([128, 128], F32, tag="o")
                nc.vector.tensor_add(o_t[:], y2_t[:], x32_t[:])
                nc.sync.dma_start(outd[:, r0:r0 + 128], o_t[:])

    nc.compile()
    return nc


def assemble(results, meta):
    N, NSH = meta["N"], meta["NSH"]
    ncores = len(results)
    out = np.empty((N, H), np.float32)
    for k in range(ncores):
        out[k * NSH:(k + 1) * NSH] = results[k]["out"][:, :NSH].T
    return out


def run_compiled(nc, in_maps, meta, ncores=8, **kw):
    res = run_bass_kernel_spmd(nc, in_maps, list(range(ncores)), **kw)
    return assemble(res.results, meta), res


def kernel(**inputs):
    inputs = {k: np.asarray(v) for k, v in inputs.items()}
    in_maps, meta = prep_inputs(**inputs)
    nc = build_program(meta)
    out, _ = run_compiled(nc, in_maps, meta)
    return out



# revision 6
# speedup vs baseline: 26.9257x; 26.9257x over previous
"""Trainium2 Bass kernel for gnn_message_passing (nn_Conv_82506321756833).

Computes, for N=50000 nodes / E=800000 edges / H=128:
    xp   = gelu(x @ W1 + b1)
    aggr = segment_sum(xp[src] * bases, dst)
    x    = x_feat + aggr
    y    = gelu(bn1(x @ W2 + b2)); y = gelu(bn2(y @ W3 + b3))
    out  = x + y

Sharding: nodes are partitioned contiguously across 8 cores (graph parallel);
each core owns its node shard and all edges whose dst lands in the shard.
Every core redundantly computes the full xp table (cheap GEMM) so the edge
gather is purely local.  The scatter-sum is done on-chip with one-hot
matmuls: edges are bucketed by 128-node destination windows (host-side sort);
for each 128-edge tile a one-hot matrix S[e, slot] selects the destination
slot; PE computes msg.T @ S accumulating into PSUM (feature-major output
feeds the FFN directly, with BN folded into W2/W3 + bias vectors).

vs the v0 baseline:
  - one-hot S is generated ON-CHIP (DVE is_equal against an iota, with a
    stride-0 broadcast of the per-edge slot index) instead of streaming a
    [NW*128, T*H] one-hot from HBM (-30 MB/core of DMA);
  - per-window variable tile counts TLO_w/THI_w (no padding to the global
    max: 828 vs 931 edge tiles per core);
  - index tables / residual features preloaded into SBUF in a few large
    DMAs; output written in 512-column batches.

The edge gather uses InstDMAGatherAnt (int16 indices, signed): each window's
edges are split into "lo" (src < 32768) and "hi" tile groups; the hi group
gathers from an offset view of the xp table.
"""

import numpy as np

import concourse.bass as bass
import concourse.bacc as bacc
import concourse.tile as tile
from concourse import mybir
from concourse.bass_utils import run_bass_kernel_spmd

H = 128
WIN = 128
SPLIT = 32768  # int16 index limit for dma_gather
BN_EPS = 1e-5
F16 = mybir.dt.float16
F32 = mybir.dt.float32
I16 = mybir.dt.int16
GELU = mybir.ActivationFunctionType.Gelu


def _wrap16(lst):
    """int16 index list (len L, L%16==0) -> [128, L//16] wrapped+replicated."""
    L = len(lst)
    m = lst.reshape(L // 16, 16).T           # [16, L/16]
    return np.ascontiguousarray(np.tile(m, (8, 1)))  # [128, L/16]


def prep_inputs(x_feat, bases, src, dst, W1, b1, W2, b2, W3, b3,
                g1, be1, m1, v1, g2, be2, m2, v2, ncores=8):
    """Host-side sharding: bucket edges by (dst window, src-range), sort,
    size each window's tile groups to its actual edge count, build per-core
    input maps."""
    N = x_feat.shape[0]
    assert N % ncores == 0
    NSH = N // ncores
    NW = (NSH + WIN - 1) // WIN
    NPAD = NW * WIN
    NA = (N + 127) // 128 * 128

    x_feat = np.asarray(x_feat, np.float32)
    bases = np.asarray(bases, np.float32)
    src = np.asarray(src, np.int64)
    dst = np.asarray(dst, np.int64)

    # xT with nodes permuted inside each 512-chunk (column b*128+p holds
    # node c0+nb*p+b) so phase A's xp writes are nb*H contiguous per
    # partition instead of nb strided 256B RMW writes.
    CH = 512
    xpad = np.zeros((NA, H), np.float32)
    xpad[:N] = x_feat
    cols = np.empty(NA, np.int64)
    for c0 in range(0, NA, CH):
        n = min(CH, NA - c0)
        nb = n // 128
        cols[c0:c0 + n] = c0 + np.arange(n).reshape(128, nb).T.ravel()
    xT = np.ascontiguousarray(xpad[cols].T.astype(np.float16))

    core_of = dst // NSH
    percore = []
    tlo_c = np.zeros((ncores, NW), np.int64)
    thi_c = np.zeros((ncores, NW), np.int64)
    for k in range(ncores):
        sel = np.nonzero(core_of == k)[0]
        ld = dst[sel] - k * NSH
        w = ld // WIN
        j = ld % WIN
        s = src[sel]
        hi = (s >= SPLIT).astype(np.int64)
        key2 = w * 2 + hi
        order = np.lexsort((s, key2))
        w, j, s, hi, key2, sel = (w[order], j[order], s[order], hi[order],
                                  key2[order], sel[order])
        cnt2 = np.bincount(key2, minlength=NW * 2)
        tlo_c[k] = (cnt2[0::2] + 127) // 128
        thi_c[k] = (cnt2[1::2] + 127) // 128
        starts2 = np.zeros(NW * 2, np.int64)
        np.cumsum(cnt2[:-1], out=starts2[1:])
        rank = np.arange(len(w)) - starts2[key2]
        percore.append((w, j, s, hi, rank, sel))

    # SPMD: one program for all cores -> per-window tile counts are the
    # max over cores
    tlo_g = tlo_c.max(axis=0)
    thi_g = thi_c.max(axis=0)
    TLOMAX = int(tlo_g.max())
    THIMAX = int(thi_g.max())
    TMAX = int((tlo_g + thi_g).max())

    w1h = np.ascontiguousarray(np.asarray(W1, np.float32).astype(np.float16))
    a1 = (np.asarray(g1, np.float32) /
          np.sqrt(np.asarray(v1, np.float32) + BN_EPS))
    a2 = (np.asarray(g2, np.float32) /
          np.sqrt(np.asarray(v2, np.float32) + BN_EPS))
    w2f = np.ascontiguousarray((np.asarray(W2, np.float32) * a1[None, :])
                               .astype(np.float16))
    w3f = np.ascontiguousarray((np.asarray(W3, np.float32) * a2[None, :])
                               .astype(np.float16))
    c2 = ((np.asarray(b2, np.float32) - np.asarray(m1, np.float32)) * a1
          + np.asarray(be1, np.float32)).astype(np.float32).reshape(H, 1)
    c3 = ((np.asarray(b3, np.float32) - np.asarray(m2, np.float32)) * a2
          + np.asarray(be2, np.float32)).astype(np.float32).reshape(H, 1)
    have_b1 = bool(np.any(np.asarray(b1)))
    b1h = np.asarray(b1, np.float32).astype(np.float16).reshape(1, H)

    ILW = max(TLOMAX, 1) * 8
    IHW = max(THIMAX, 1) * 8
    in_maps = []
    for k in range(ncores):
        w, j, s, hi, rank, sel = percore[k]
        # tile index within the window: lo tiles [0, tlo_g[w]), hi after
        t_of = np.where(hi == 1, tlo_g[w] + rank // 128, rank // 128)
        p_of = rank % 128
        rows = w * 128 + p_of

        basd = np.zeros((NW * 128, TMAX * H), np.float16)
        basd[rows[:, None],
             t_of[:, None] * H + np.arange(H)[None, :]] = \
            bases[sel].astype(np.float16)

        jdm = np.full((NW * 128, TMAX), 128.0, np.float16)
        jdm[rows, t_of] = j.astype(np.float16)

        # gather index lists, wrap16 layout: list element i of window wi
        # lands at row wi*128 + i%16 (replicated to all 8 16-partition
        # groups), column i//16
        ilod = np.zeros((NW * 128, ILW), np.int16)
        ihid = np.zeros((NW * 128, IHW), np.int16)
        lo_m = hi == 0
        ilod[w[lo_m] * 128 + rank[lo_m] % 16, rank[lo_m] // 16] = \
            s[lo_m].astype(np.int16)
        hi_m = hi == 1
        ihid[w[hi_m] * 128 + rank[hi_m] % 16, rank[hi_m] // 16] = \
            (s[hi_m] - SPLIT).astype(np.int16)
        for arr in (ilod, ihid):
            blk = arr.reshape(NW, 8, 16, -1)
            blk[:, 1:] = blk[:, :1]

        xfm = np.zeros((H, NPAD), np.float32)
        xfm[:, :NSH] = x_feat[k * NSH:(k + 1) * NSH].T
        maps = dict(xT=xT, basd=basd, jdm=jdm, ilod=ilod, xfm=xfm,
                    w1=w1h, w2=w2f, w3=w3f, c2=c2, c3=c3)
        if THIMAX:
            maps["ihid"] = ihid
        if have_b1:
            maps["b1"] = b1h
        in_maps.append(maps)

    meta = dict(N=N, NSH=NSH, NW=NW, NPAD=NPAD, NA=NA,
                TLOMAX=TLOMAX, THIMAX=THIMAX, TMAX=TMAX,
                tlo=tlo_g.tolist(), thi=thi_g.tolist(), have_b1=have_b1)
    return in_maps, meta


def build_program(meta, ncores=8, act=GELU):
    NA, NW, NPAD = meta["NA"], meta["NW"], meta["NPAD"]
    TLOMAX, THIMAX, TMAX = meta["TLOMAX"], meta["THIMAX"], meta["TMAX"]
    tlo, thi = meta["tlo"], meta["thi"]
    have_b1 = meta["have_b1"]
    ILW = max(TLOMAX, 1) * 8
    IHW = max(THIMAX, 1) * 8

    nc = bacc.Bacc("TRN2", target_bir_lowering=False, debug=False,
                   num_devices=ncores)
    xT = nc.dram_tensor("xT", [H, NA], F16, kind="ExternalInput").ap()
    xfm = nc.dram_tensor("xfm", [H, NPAD], F32, kind="ExternalInput").ap()
    basd = nc.dram_tensor("basd", [NW * 128, TMAX * H], F16,
                          kind="ExternalInput").ap()
    jdm = nc.dram_tensor("jdm", [NW * 128, TMAX], F16,
                         kind="ExternalInput").ap()
    ilod = nc.dram_tensor("ilod", [NW * 128, ILW], I16,
                          kind="ExternalInput").ap()
    ihid = (nc.dram_tensor("ihid", [NW * 128, IHW], I16,
                           kind="ExternalInput").ap() if THIMAX else None)
    w1 = nc.dram_tensor("w1", [H, H], F16, kind="ExternalInput").ap()
    w2 = nc.dram_tensor("w2", [H, H], F16, kind="ExternalInput").ap()
    w3 = nc.dram_tensor("w3", [H, H], F16, kind="ExternalInput").ap()
    c2 = nc.dram_tensor("c2", [H, 1], F32, kind="ExternalInput").ap()
    c3 = nc.dram_tensor("c3", [H, 1], F32, kind="ExternalInput").ap()
    b1 = (nc.dram_tensor("b1", [1, H], F16, kind="ExternalInput").ap()
          if have_b1 else None)
    outd = nc.dram_tensor("out", [H, NPAD], F32, kind="ExternalOutput").ap()
    xp_h = nc.dram_tensor("xp", [NA, H], F16)
    xp_full = xp_h.ap()

    OB = 4  # output windows batched per DMA

    with tile.TileContext(nc) as tc:
        with (
            tc.tile_pool(name="const", bufs=1) as cpool,
            tc.tile_pool(name="pre", bufs=1) as prep,
            tc.tile_pool(name="xa", bufs=3) as xa,
            tc.tile_pool(name="xo", bufs=3) as xo,
            tc.tile_pool(name="pa", bufs=2, space="PSUM") as pa,
            tc.tile_pool(name="bas", bufs=3) as basp,
            tc.tile_pool(name="gat", bufs=3) as gatp,
            tc.tile_pool(name="st", bufs=3) as stp,
            tc.tile_pool(name="ffn", bufs=2) as ffnp,
            tc.tile_pool(name="ob", bufs=2) as obp,
            tc.tile_pool(name="pag", bufs=2, space="PSUM") as pag,
            tc.tile_pool(name="pffn", bufs=2, space="PSUM") as pffn,
        ):
            # constants
            w1t = cpool.tile([H, H], F16, tag="w1")
            nc.sync.dma_start(w1t[:], w1[:])
            w2t = cpool.tile([H, H], F16, tag="w2")
            nc.sync.dma_start(w2t[:], w2[:])
            w3t = cpool.tile([H, H], F16, tag="w3")
            nc.sync.dma_start(w3t[:], w3[:])
            c2t = cpool.tile([H, 1], F32, tag="c2")
            nc.sync.dma_start(c2t[:], c2[:])
            c3t = cpool.tile([H, 1], F32, tag="c3")
            nc.sync.dma_start(c3t[:], c3[:])
            iota_t = cpool.tile([128, TMAX * 128], F16, tag="iota")
            nc.gpsimd.iota(iota_t[:], pattern=[[0, TMAX], [1, 128]], base=0,
                           channel_multiplier=0,
                           allow_small_or_imprecise_dtypes=True)
            if have_b1:
                b1t = cpool.tile([1, H], F16, tag="b1")
                nc.sync.dma_start(b1t[:], b1[:])
                onest = cpool.tile([1, H], F16, tag="ones")
                nc.gpsimd.memset(onest[:], 1.0)

            # preloads: slot indices, gather indices, residual features
            jd_sb = prep.tile([128, NW * TMAX], F16, tag="jd")
            nc.scalar.dma_start(
                jd_sb[:],
                bass.AP(jdm.tensor, 0,
                        [[TMAX, 128], [128 * TMAX, NW], [1, TMAX]]))
            il_sb = prep.tile([128, NW * ILW], I16, tag="il")
            nc.scalar.dma_start(
                il_sb[:],
                bass.AP(ilod.tensor, 0,
                        [[ILW, 128], [128 * ILW, NW], [1, ILW]]))
            if THIMAX:
                ih_sb = prep.tile([128, NW * IHW], I16, tag="ih")
                nc.scalar.dma_start(
                    ih_sb[:],
                    bass.AP(ihid.tensor, 0,
                            [[IHW, 128], [128 * IHW, NW], [1, IHW]]))
            xf_sb = prep.tile([128, NPAD], F32, tag="xf")
            nc.sync.dma_start(xf_sb[:], xfm[:])

            # ---- Phase A: xp = gelu(x @ W1 [+ b1]), node-major fp16.
            # xT columns are chunk-permuted on host (column b*128+p = node
            # c0+nb*p+b) so each partition writes nb*H contiguous elements.
            CH = 512
            for c0 in range(0, NA, CH):
                cols = min(CH, NA - c0)
                nb = cols // 128
                xt_t = xa.tile([H, CH], F16, tag="xa")
                nc.sync.dma_start(xt_t[:, :cols], xT[:, c0:c0 + cols])
                ps = pa.tile([128, CH], F32, tag="pa")
                for b in range(nb):
                    nc.tensor.matmul(
                        ps[:, b * 128:(b + 1) * 128],
                        xt_t[:, b * 128:(b + 1) * 128],
                        w1t[:],
                        start=True, stop=not have_b1)
                    if have_b1:
                        nc.tensor.matmul(
                            ps[:, b * 128:(b + 1) * 128],
                            onest[:1, :], b1t[:1, :],
                            start=False, stop=True)
                xo_t = xo.tile([128, CH], F16, tag="xo")
                nc.scalar.activation(xo_t[:, :cols], ps[:, :cols], act)
                dst_ap = bass.AP(xp_h, c0 * H,
                                 [[nb * H, 128], [1, nb * H]])
                nc.sync.dma_start(dst_ap, xo_t[:, :cols])

            # ---- Phase B: gather, multiply, one-hot scatter, FFN ----
            o_acc = None
            for w in range(NW):
                r0 = w * 128
                TLO, THI = tlo[w], thi[w]
                T = TLO + THI
                EPW = T * H
                bas_t = basp.tile([128, TMAX * H], F16, tag="bas")
                nc.sync.dma_start(bas_t[:, :EPW], basd[r0:r0 + 128, :EPW])

                g_t = gatp.tile([128, TMAX * H], F16, tag="gat")
                g3 = g_t[:, :EPW].rearrange("p (t f) -> p t f", f=H)
                if TLO:
                    nc.gpsimd.dma_gather(
                        g3[:, 0:TLO, :], xp_full,
                        il_sb[:, w * ILW:w * ILW + TLO * 8],
                        TLO * 128, TLO * 128, H, single_packet=False)
                if THI:
                    nc.gpsimd.dma_gather(
                        g3[:, TLO:T, :], xp_full[SPLIT:NA, :],
                        ih_sb[:, w * IHW:w * IHW + THI * 8],
                        THI * 128, THI * 128, H, single_packet=False)
                msg_t = gatp.tile([128, TMAX * H], F16, tag="msg")
                nc.vector.tensor_mul(msg_t[:, :EPW], g_t[:, :EPW],
                                     bas_t[:, :EPW])

                # one-hot S[e, t*128+n] = (jd[e, t] == n), via stride-0
                # broadcast of the slot index against a repeated iota
                s_t = stp.tile([128, TMAX * 128], F16, tag="s")
                jsl = jd_sb[:, w * TMAX:w * TMAX + T]
                jb = bass.AP(jd_sb.tensor, jsl.offset,
                             [list(jsl.ap[0]), [1, T], [0, 128]])
                nc.vector.tensor_tensor(
                    s_t[:, :T * 128].rearrange("p (t n) -> p t n", n=128),
                    iota_t[:, :T * 128].rearrange("p (t n) -> p t n", n=128),
                    jb, mybir.AluOpType.is_equal)

                ps_ag = pag.tile([128, 128], F32, tag="pag")
                for t in range(T):
                    nc.tensor.matmul(
                        ps_ag[:],
                        msg_t[:, t * H:(t + 1) * H],
                        s_t[:, t * 128:(t + 1) * 128],
                        start=(t == 0), stop=(t == T - 1))

                x32_t = ffnp.tile([128, 128], F32, tag="x32")
                nc.vector.tensor_add(x32_t[:], ps_ag[:],
                                     xf_sb[:, r0:r0 + 128])
                x16_t = ffnp.tile([128, 128], F16, tag="x16")
                nc.vector.tensor_copy(x16_t[:], x32_t[:])

                ps2 = pffn.tile([128, 128], F32, tag="pffn")
                nc.tensor.matmul(ps2[:], w2t[:], x16_t[:],
                                 start=True, stop=True)
                y1_t = ffnp.tile([128, 128], F16, tag="y1")
                nc.scalar.activation(y1_t[:], ps2[:], act, bias=c2t[:, 0:1])
                ps3 = pffn.tile([128, 128], F32, tag="pffn")
                nc.tensor.matmul(ps3[:], w3t[:], y1_t[:],
                                 start=True, stop=True)
                y2_t = ffnp.tile([128, 128], F32, tag="y2")
                nc.scalar.activation(y2_t[:], ps3[:], act, bias=c3t[:, 0:1])

                if w % OB == 0:
                    o_acc = obp.tile([128, OB * 128], F32, tag="oacc")
                oc = (w % OB) * 128
                nc.vector.tensor_add(o_acc[:, oc:oc + 128], y2_t[:],
                                     x32_t[:])
                if w % OB == OB - 1 or w == NW - 1:
                    n_in = (w % OB) + 1
                    nc.sync.dma_start(
                        outd[:, (w - n_in + 1) * 128:(w + 1) * 128],
                        o_acc[:, :n_in * 128])

    nc.compile()
    return nc


def assemble(results, meta):
    N, NSH = meta["N"], meta["NSH"]
    ncores = len(results)
    out = np.empty((N, H), np.float32)
    for k in range(ncores):
        out[k * NSH:(k + 1) * NSH] = results[k]["out"][:, :NSH].T
    return out


def run_compiled(nc, in_maps, meta, ncores=8, **kw):
    res = run_bass_kernel_spmd(nc, in_maps, list(range(ncores)), **kw)
    return assemble(res.results, meta), res


def kernel(**inputs):
    inputs = {k: np.asarray(v) for k, v in inputs.items()}
    in_maps, meta = prep_inputs(**inputs)
    nc = build_program(meta)
    out, _ = run_compiled(nc, in_maps, meta)
    return out


# revision 15
# speedup vs baseline: 494.5906x; 18.3687x over previous
"""Trainium2 Bass kernel for gnn_message_passing (nn_Conv_82506321756833).

Computes, for N=50000 nodes / E=800000 edges / H=128:
    xp   = gelu(x @ W1 + b1)
    aggr = segment_sum(xp[src] * bases, dst)
    x    = x_feat + aggr
    y    = gelu(bn1(x @ W2 + b2)); y = gelu(bn2(y @ W3 + b3))
    out  = x + y

Sharding: nodes are partitioned contiguously across 8 cores (graph parallel);
each core owns its node shard and all edges whose dst lands in the shard.
Every core redundantly computes the full xp table (cheap GEMM) so the edge
gather is purely local.  The scatter-sum is done on-chip with one-hot
matmuls: edges are bucketed by 128-node destination windows (host-side sort);
for each 128-edge tile a one-hot matrix S[e, slot] selects the destination
slot; PE computes msg.T @ S accumulating into PSUM (feature-major output
feeds the FFN directly, with BN folded into W2/W3 + bias vectors).

vs the v0 baseline:
  - one-hot S is generated ON-CHIP (DVE is_equal against an iota, with a
    stride-0 broadcast of the per-edge slot index) instead of streaming a
    [NW*128, T*H] one-hot from HBM (-30 MB/core of DMA);
  - per-window variable tile counts TLO_w/THI_w (no padding to the global
    max: 828 vs 931 edge tiles per core);
  - index tables / residual features preloaded into SBUF in a few large
    DMAs; output written in 512-column batches.

The edge gather uses InstDMAGatherAnt (int16 indices, signed): each window's
edges are split into "lo" (src < 32768) and "hi" tile groups; the hi group
gathers from an offset view of the xp table.
"""

import numpy as np

import concourse.bass as bass
import concourse.bacc as bacc
import concourse.tile as tile
from concourse import mybir
from concourse.bass_utils import run_bass_kernel_spmd

H = 128
WIN = 128
SPLIT = 32768  # int16 index limit for dma_gather
BN_EPS = 1e-5
F16 = mybir.dt.float16
F32 = mybir.dt.float32
I16 = mybir.dt.int16
GELU = mybir.ActivationFunctionType.Gelu


def _wrap16(lst):
    """int16 index list (len L, L%16==0) -> [128, L//16] wrapped+replicated."""
    L = len(lst)
    m = lst.reshape(L // 16, 16).T           # [16, L/16]
    return np.ascontiguousarray(np.tile(m, (8, 1)))  # [128, L/16]


def _layout(meta):
    """Element offsets of the logical tensors inside the two input blobs.
    blob16 (fp16 elements, int16 index data bitcast in place); blob32 (fp32).
    Packing everything into two ExternalInputs matters: per-dispatch overhead
    grows ~50 us per input tensor (measured), dwarfing the kernel itself."""
    NA, NW, NPAD = meta["NA"], meta["NW"], meta["NPAD"]
    TMAX = meta["TMAX"]
    ILW = max(meta["TLOMAX"], 1) * 8
    IHW = max(meta["THIMAX"], 1) * 8
    off = {}
    o = 0
    for name, ln in (("xT", H * NA), ("basd", NW * 128 * TMAX * H),
                     ("jdm", NW * 128 * TMAX), ("ilod", NW * 128 * ILW),
                     ("ihid", NW * 128 * IHW), ("w1", H * H), ("w2", H * H),
                     ("w3", H * H), ("b1", H)):
        off[name] = o
        o += ln
    off["_len16"] = o
    o32 = 0
    for name, ln in (("xfm", H * NPAD), ("c2", H), ("c3", H)):
        off[name] = o32
        o32 += ln
    off["_len32"] = o32
    off["ILW"], off["IHW"] = ILW, IHW
    return off


def prep_inputs(x_feat, bases, src, dst, W1, b1, W2, b2, W3, b3,
                g1, be1, m1, v1, g2, be2, m2, v2, ncores=8):
    """Host-side sharding: bucket edges by (dst window, src-range), sort,
    size each window's tile groups to its actual edge count, build per-core
    input maps."""
    N = x_feat.shape[0]
    assert N % ncores == 0
    NSH = N // ncores
    NW = (NSH + WIN - 1) // WIN
    NPAD = NW * WIN
    NA = (N + 127) // 128 * 128

    x_feat = np.asarray(x_feat, np.float32)
    bases = np.asarray(bases, np.float32)
    src = np.asarray(src, np.int64)
    dst = np.asarray(dst, np.int64)

    # xT with nodes permuted inside each 512-chunk (column b*128+p holds
    # node c0+nb*p+b) so phase A's xp writes are nb*H contiguous per
    # partition instead of nb strided 256B RMW writes.
    CH = 512
    xpad = np.zeros((NA, H), np.float32)
    xpad[:N] = x_feat
    cols = np.empty(NA, np.int64)
    for c0 in range(0, NA, CH):
        n = min(CH, NA - c0)
        nb = n // 128
        cols[c0:c0 + n] = c0 + np.arange(n).reshape(128, nb).T.ravel()
    xT = np.ascontiguousarray(xpad[cols].T.astype(np.float16))

    core_of = dst // NSH
    percore = []
    tlo_c = np.zeros((ncores, NW), np.int64)
    thi_c = np.zeros((ncores, NW), np.int64)
    for k in range(ncores):
        sel = np.nonzero(core_of == k)[0]
        ld = dst[sel] - k * NSH
        w = ld // WIN
        j = ld % WIN
        s = src[sel]
        hi = (s >= SPLIT).astype(np.int64)
        key2 = w * 2 + hi
        order = np.lexsort((s, key2))
        w, j, s, hi, key2, sel = (w[order], j[order], s[order], hi[order],
                                  key2[order], sel[order])
        cnt2 = np.bincount(key2, minlength=NW * 2)
        tlo_c[k] = (cnt2[0::2] + 127) // 128
        thi_c[k] = (cnt2[1::2] + 127) // 128
        starts2 = np.zeros(NW * 2, np.int64)
        np.cumsum(cnt2[:-1], out=starts2[1:])
        rank = np.arange(len(w)) - starts2[key2]
        percore.append((w, j, s, hi, rank, sel))

    # SPMD: one program for all cores -> per-window tile counts are the
    # max over cores
    tlo_g = tlo_c.max(axis=0)
    thi_g = thi_c.max(axis=0)
    TLOMAX = int(tlo_g.max())
    THIMAX = int(thi_g.max())
    TMAX = int((tlo_g + thi_g).max())

    w1h = np.ascontiguousarray(np.asarray(W1, np.float32).astype(np.float16))
    a1 = (np.asarray(g1, np.float32) /
          np.sqrt(np.asarray(v1, np.float32) + BN_EPS))
    a2 = (np.asarray(g2, np.float32) /
          np.sqrt(np.asarray(v2, np.float32) + BN_EPS))
    w2f = np.ascontiguousarray((np.asarray(W2, np.float32) * a1[None, :])
                               .astype(np.float16))
    w3f = np.ascontiguousarray((np.asarray(W3, np.float32) * a2[None, :])
                               .astype(np.float16))
    c2 = ((np.asarray(b2, np.float32) - np.asarray(m1, np.float32)) * a1
          + np.asarray(be1, np.float32)).astype(np.float32).reshape(H, 1)
    c3 = ((np.asarray(b3, np.float32) - np.asarray(m2, np.float32)) * a2
          + np.asarray(be2, np.float32)).astype(np.float32).reshape(H, 1)
    have_b1 = bool(np.any(np.asarray(b1)))
    b1h = np.asarray(b1, np.float32).astype(np.float16).reshape(1, H)

    ILW = max(TLOMAX, 1) * 8
    IHW = max(THIMAX, 1) * 8
    in_maps = []
    for k in range(ncores):
        w, j, s, hi, rank, sel = percore[k]
        # tile index within the window: lo tiles [0, tlo_g[w]), hi after
        t_of = np.where(hi == 1, tlo_g[w] + rank // 128, rank // 128)
        p_of = rank % 128
        rows = w * 128 + p_of

        basd = np.zeros((NW * 128, TMAX * H), np.float16)
        basd[rows[:, None],
             t_of[:, None] * H + np.arange(H)[None, :]] = \
            bases[sel].astype(np.float16)

        jdm = np.full((NW * 128, TMAX), 128.0, np.float16)
        jdm[rows, t_of] = j.astype(np.float16)

        # gather index lists, wrap16 layout: list element i of window wi
        # lands at row wi*128 + i%16 (replicated to all 8 16-partition
        # groups), column i//16
        ilod = np.zeros((NW * 128, ILW), np.int16)
        ihid = np.zeros((NW * 128, IHW), np.int16)
        lo_m = hi == 0
        ilod[w[lo_m] * 128 + rank[lo_m] % 16, rank[lo_m] // 16] = \
            s[lo_m].astype(np.int16)
        hi_m = hi == 1
        ihid[w[hi_m] * 128 + rank[hi_m] % 16, rank[hi_m] // 16] = \
            (s[hi_m] - SPLIT).astype(np.int16)
        for arr in (ilod, ihid):
            blk = arr.reshape(NW, 8, 16, -1)
            blk[:, 1:] = blk[:, :1]

        xfm = np.zeros((H, NPAD), np.float32)
        xfm[:, :NSH] = x_feat[k * NSH:(k + 1) * NSH].T
        in_maps.append(dict(basd=basd, jdm=jdm, ilod=ilod, ihid=ihid,
                            xfm=xfm))

    meta = dict(N=N, NSH=NSH, NW=NW, NPAD=NPAD, NA=NA,
                TLOMAX=TLOMAX, THIMAX=THIMAX, TMAX=TMAX,
                tlo=tlo_g.tolist(), thi=thi_g.tolist(), have_b1=have_b1)

    # pack everything into two blobs (see _layout)
    off = _layout(meta)
    for k in range(ncores):
        m = in_maps[k]
        b16 = np.zeros(off["_len16"], np.float16)
        for name, arr in (("xT", xT), ("basd", m.pop("basd")),
                          ("jdm", m.pop("jdm")),
                          ("ilod", m.pop("ilod").view(np.float16)),
                          ("ihid", m.pop("ihid").view(np.float16)),
                          ("w1", w1h), ("w2", w2f), ("w3", w3f),
                          ("b1", b1h)):
            fl = arr.ravel()
            b16[off[name]:off[name] + fl.size] = fl
        b32 = np.zeros(off["_len32"], np.float32)
        for name, arr in (("xfm", m.pop("xfm")), ("c2", c2), ("c3", c3)):
            fl = arr.ravel()
            b32[off[name]:off[name] + fl.size] = fl
        in_maps[k] = dict(b16=b16.reshape(1, -1), b32=b32.reshape(1, -1))
    return in_maps, meta


def build_program(meta, ncores=8, act=GELU, ablate=()):
    ablate = frozenset(ablate)  # timing-attribution knob; empty in production
    NA, NW, NPAD = meta["NA"], meta["NW"], meta["NPAD"]
    TLOMAX, THIMAX, TMAX = meta["TLOMAX"], meta["THIMAX"], meta["TMAX"]
    tlo, thi = meta["tlo"], meta["thi"]
    have_b1 = meta["have_b1"]
    ILW = max(TLOMAX, 1) * 8
    IHW = max(THIMAX, 1) * 8

    off = _layout(meta)
    nc = bacc.Bacc("TRN2", target_bir_lowering=False, debug=False,
                   num_devices=ncores)
    b16h = nc.dram_tensor("b16", [1, off["_len16"]], F16,
                          kind="ExternalInput")
    b32h = nc.dram_tensor("b32", [1, off["_len32"]], F32,
                          kind="ExternalInput")

    def v16(name, pattern, extra=0):
        return bass.AP(b16h, off[name] + extra, pattern)

    def v32(name, pattern, extra=0):
        return bass.AP(b32h, off[name] + extra, pattern)

    xT = lambda c0, cols: v16("xT", [[NA, 128], [1, cols]], c0)
    basd_w = lambda r0, cols: v16("basd", [[TMAX * H, 128], [1, cols]],
                                  r0 * TMAX * H)
    jdm_all = v16("jdm", [[TMAX, 128], [128 * TMAX, NW], [1, TMAX]])
    ilod_all = v16("ilod", [[ILW, 128], [128 * ILW, NW], [1, ILW]]) \
        .bitcast(I16)
    ihid_all = v16("ihid", [[IHW, 128], [128 * IHW, NW], [1, IHW]]) \
        .bitcast(I16)
    w1 = v16("w1", [[H, H], [1, H]])
    w2 = v16("w2", [[H, H], [1, H]])
    w3 = v16("w3", [[H, H], [1, H]])
    b1 = v16("b1", [[H, 1], [1, H]])
    xfm = v32("xfm", [[NPAD, 128], [1, NPAD]])
    c2 = v32("c2", [[1, H], [1, 1]])
    c3 = v32("c3", [[1, H], [1, 1]])
    outd = nc.dram_tensor("out", [H, NPAD], F32, kind="ExternalOutput").ap()
    xp_h = nc.dram_tensor("xp", [NA, H], F16)
    xp_full = xp_h.ap()

    OB = 4  # output windows batched per DMA

    with tile.TileContext(nc) as tc:
        with (
            tc.tile_pool(name="const", bufs=1) as cpool,
            tc.tile_pool(name="pre", bufs=1) as prep,
            tc.tile_pool(name="xa", bufs=3) as xa,
            tc.tile_pool(name="xo", bufs=3) as xo,
            tc.tile_pool(name="pa", bufs=2, space="PSUM") as pa,
            tc.tile_pool(name="bas", bufs=3) as basp,
            tc.tile_pool(name="gat", bufs=3) as gatp,
            tc.tile_pool(name="st", bufs=3) as stp,
            tc.tile_pool(name="ffn", bufs=2) as ffnp,
            tc.tile_pool(name="ob", bufs=2) as obp,
            tc.tile_pool(name="pag", bufs=2, space="PSUM") as pag,
            tc.tile_pool(name="pffn", bufs=2, space="PSUM") as pffn,
        ):
            # constants
            w1t = cpool.tile([H, H], F16, tag="w1")
            nc.sync.dma_start(w1t[:], w1)
            w2t = cpool.tile([H, H], F16, tag="w2")
            nc.sync.dma_start(w2t[:], w2)
            w3t = cpool.tile([H, H], F16, tag="w3")
            nc.sync.dma_start(w3t[:], w3)
            c2t = cpool.tile([H, 1], F32, tag="c2")
            nc.sync.dma_start(c2t[:], c2)
            c3t = cpool.tile([H, 1], F32, tag="c3")
            nc.sync.dma_start(c3t[:], c3)
            iota_t = cpool.tile([128, TMAX * 128], F16, tag="iota")
            nc.gpsimd.iota(iota_t[:], pattern=[[0, TMAX], [1, 128]], base=0,
                           channel_multiplier=0,
                           allow_small_or_imprecise_dtypes=True)
            if have_b1:
                b1t = cpool.tile([1, H], F16, tag="b1")
                nc.sync.dma_start(b1t[:], b1)
                onest = cpool.tile([1, H], F16, tag="ones")
                nc.gpsimd.memset(onest[:], 1.0)

            # preloads: slot indices, gather indices, residual features
            jd_sb = prep.tile([128, NW * TMAX], F16, tag="jd")
            nc.scalar.dma_start(jd_sb[:], jdm_all)
            il_sb = prep.tile([128, NW * ILW], I16, tag="il")
            nc.scalar.dma_start(il_sb[:], ilod_all)
            if THIMAX:
                ih_sb = prep.tile([128, NW * IHW], I16, tag="ih")
                nc.scalar.dma_start(ih_sb[:], ihid_all)
            if "onehot_ts" in ablate:
                jd32_sb = prep.tile([128, NW * TMAX], F32, tag="jd32")
                nc.vector.tensor_copy(jd32_sb[:], jd_sb[:])
            xf_sb = prep.tile([128, NPAD], F32, tag="xf")
            nc.sync.dma_start(xf_sb[:], xfm)

            # ---- Phase A: xp = gelu(x @ W1 [+ b1]), node-major fp16.
            # xT columns are chunk-permuted on host (column b*128+p = node
            # c0+nb*p+b) so each partition writes nb*H contiguous elements.
            CH = 512
            for c0 in range(0, NA, CH) if "phaseA" not in ablate else []:
                cols = min(CH, NA - c0)
                nb = cols // 128
                xt_t = xa.tile([H, CH], F16, tag="xa")
                nc.sync.dma_start(xt_t[:, :cols], xT(c0, cols))
                ps = pa.tile([128, CH], F32, tag="pa")
                for b in range(nb):
                    nc.tensor.matmul(
                        ps[:, b * 128:(b + 1) * 128],
                        xt_t[:, b * 128:(b + 1) * 128],
                        w1t[:],
                        start=True, stop=not have_b1)
                    if have_b1:
                        nc.tensor.matmul(
                            ps[:, b * 128:(b + 1) * 128],
                            onest[:1, :], b1t[:1, :],
                            start=False, stop=True)
                xo_t = xo.tile([128, CH], F16, tag="xo")
                nc.scalar.activation(xo_t[:, :cols], ps[:, :cols], act)
                dst_ap = bass.AP(xp_h, c0 * H,
                                 [[nb * H, 128], [1, nb * H]])
                nc.sync.dma_start(dst_ap, xo_t[:, :cols])

            # ---- Phase B: gather, multiply, one-hot scatter, FFN ----
            o_acc = None
            for w in range(NW):
                r0 = w * 128
                TLO, THI = tlo[w], thi[w]
                T = TLO + THI
                EPW = T * H
                if "bas" not in ablate:
                    bas_t = basp.tile([128, TMAX * H], F16, tag="bas")
                    nc.sync.dma_start(bas_t[:, :EPW], basd_w(r0, EPW))

                if "gather" not in ablate:
                    g_t = gatp.tile([128, TMAX * H], F16, tag="gat")
                    g3 = g_t[:, :EPW].rearrange("p (t f) -> p t f", f=H)
                    if TLO:
                        nc.gpsimd.dma_gather(
                            g3[:, 0:TLO, :], xp_full,
                            il_sb[:, w * ILW:w * ILW + TLO * 8],
                            TLO * 128, TLO * 128, H, single_packet=False)
                    if THI:
                        nc.gpsimd.dma_gather(
                            g3[:, TLO:T, :], xp_full[SPLIT:NA, :],
                            ih_sb[:, w * IHW:w * IHW + THI * 8],
                            THI * 128, THI * 128, H, single_packet=False)
                if "mul" not in ablate:
                    msg_t = gatp.tile([128, TMAX * H], F16, tag="msg")
                    nc.vector.tensor_mul(msg_t[:, :EPW], g_t[:, :EPW],
                                         bas_t[:, :EPW])

                # one-hot S[e, t*128+n] = (jd[e, t] == n), via stride-0
                # broadcast of the slot index against a repeated iota
                if "onehot" not in ablate:
                    s_t = stp.tile([128, TMAX * 128], F16, tag="s")
                    if "onehot_ts" in ablate:
                        for t in range(T):
                            nc.vector.tensor_scalar(
                                s_t[:, t * 128:(t + 1) * 128],
                                iota_t[:, t * 128:(t + 1) * 128],
                                jd32_sb[:, w * TMAX + t:w * TMAX + t + 1],
                                None, mybir.AluOpType.is_equal)
                    else:
                        jsl = jd_sb[:, w * TMAX:w * TMAX + T]
                        jb = bass.AP(jd_sb.tensor, jsl.offset,
                                     [list(jsl.ap[0]), [1, T], [0, 128]])
                        nc.vector.tensor_tensor(
                            s_t[:, :T * 128].rearrange("p (t n) -> p t n", n=128),
                            iota_t[:, :T * 128].rearrange("p (t n) -> p t n", n=128),
                            jb, mybir.AluOpType.is_equal)

                if "scatter" not in ablate:
                    ps_ag = pag.tile([128, 128], F32, tag="pag")
                    for t in range(T):
                        nc.tensor.matmul(
                            ps_ag[:],
                            msg_t[:, t * H:(t + 1) * H],
                            s_t[:, t * 128:(t + 1) * 128],
                            start=(t == 0), stop=(t == T - 1))

                if "ffn" in ablate:
                    continue
                x32_t = ffnp.tile([128, 128], F32, tag="x32")
                nc.vector.tensor_add(x32_t[:], ps_ag[:],
                                     xf_sb[:, r0:r0 + 128])
                x16_t = ffnp.tile([128, 128], F16, tag="x16")
                nc.vector.tensor_copy(x16_t[:], x32_t[:])

                ps2 = pffn.tile([128, 128], F32, tag="pffn")
                nc.tensor.matmul(ps2[:], w2t[:], x16_t[:],
                                 start=True, stop=True)
                y1_t = ffnp.tile([128, 128], F16, tag="y1")
                nc.scalar.activation(y1_t[:], ps2[:], act, bias=c2t[:, 0:1])
                ps3 = pffn.tile([128, 128], F32, tag="pffn")
                nc.tensor.matmul(ps3[:], w3t[:], y1_t[:],
                                 start=True, stop=True)
                y2_t = ffnp.tile([128, 128], F32, tag="y2")
                nc.scalar.activation(y2_t[:], ps3[:], act, bias=c3t[:, 0:1])

                if w % OB == 0:
                    o_acc = obp.tile([128, OB * 128], F32, tag="oacc")
                oc = (w % OB) * 128
                nc.vector.tensor_add(o_acc[:, oc:oc + 128], y2_t[:],
                                     x32_t[:])
                if w % OB == OB - 1 or w == NW - 1:
                    n_in = (w % OB) + 1
                    nc.sync.dma_start(
                        outd[:, (w - n_in + 1) * 128:(w + 1) * 128],
                        o_acc[:, :n_in * 128])

    nc.compile()
    return nc


def assemble(results, meta):
    N, NSH = meta["N"], meta["NSH"]
    ncores = len(results)
    out = np.empty((N, H), np.float32)
    for k in range(ncores):
        out[k * NSH:(k + 1) * NSH] = results[k]["out"][:, :NSH].T
    return out


def run_compiled(nc, in_maps, meta, ncores=8, **kw):
    res = run_bass_kernel_spmd(nc, in_maps, list(range(ncores)), **kw)
    return assemble(res.results, meta), res


def kernel(**inputs):
    inputs = {k: np.asarray(v) for k, v in inputs.items()}
    in_maps, meta = prep_inputs(**inputs)
    nc = build_program(meta)
    out, _ = run_compiled(nc, in_maps, meta)
    return out
